# revision 6
# baseline (speedup 1.0000x reference)
"""Trainium2 Bass kernel for nn_ETypePromptModel: logits = einsum('bpd,cpd->bc').

Equivalent to X @ W.T with X=[B, L*D]=[16384, 256], W=[C, L*D]=[4096, 256].
Data-parallel over B across 8 NeuronCores; label2embed replicated.

Per-core plan (B_LOC=2048):
  - Load X (1 DMA, 2 MB) and W (4 DMAs, 4 MB) up front at line rate.
  - PE-transpose X (32 tiles) and W (64 tiles) into K-major float32r SBUF
    layout; 4 transposes batched per PSUM bank -> one [128,512] copy each.
  - 256 float32r matmuls ([128k x 128b] stationary, [128k x 512c] moving),
    K=256 accumulated over 2 PSUM passes, 6 PSUM banks in rotation.
  - PSUM -> SBUF copies alternate Vector/Scalar engines; 16 x 2MB HWDGE
    DMA writes of the [2048, 4096] fp32 output slice.
"""

import sys

import numpy as np

sys.path.insert(0, "/opt/trn_rl_repo")

B, C, L, D = 16384, 4096, 2, 128
N_CORES = 8
B_LOC = B // N_CORES  # 2048
P = 128
N_TILE = 512  # moving free dim per matmul
M_TILES = B_LOC // P  # 16
N_TILES = C // N_TILE  # 8
C_TILES = C // P  # 32
N_GROUP = 4  # matmul accumulation group width (PSUM banks)
W_CHUNKS = 4  # input-load chunks for W
C_HALF = C // 2

_CACHE = {}
PROFILE = False
TRACE_ALL_CORES = False
LAST_RESULT = None


def _build():
    import concourse.mybir as mybir
    import concourse.tile as tile
    from concourse import bacc
    from concourse.masks import make_identity

    f32 = mybir.dt.float32
    f32r = mybir.dt.float32r

    nc = bacc.Bacc(
        "TRN2",
        target_bir_lowering=False,
        debug=False,
        enable_asserts=False,
        num_devices=N_CORES,
    )

    x_dram = nc.dram_tensor("batchs", [B_LOC, L, D], f32, kind="ExternalInput").ap()
    w_dram = nc.dram_tensor("label2embed", [C, L, D], f32, kind="ExternalInput").ap()
    out_dram = nc.dram_tensor("out", [B_LOC, C], f32, kind="ExternalOutput").ap()

    with tile.TileContext(nc) as tc:
        with (
            tc.tile_pool(name="const", bufs=1) as const_pool,
            tc.tile_pool(name="big", bufs=1) as big_pool,
            tc.tile_pool(name="osb", bufs=4) as out_pool,
            tc.tile_pool(name="pst", bufs=3, space="PSUM") as psum_t,
            tc.tile_pool(name="psm", bufs=5, space="PSUM") as psum_mm,
        ):
            ident = const_pool.tile([P, P], f32, name="ident")
            make_identity(nc, ident)

            _cp = [0]

            def copy(out_ap, in_ap):
                if _cp[0] % 2 == 0:
                    nc.vector.tensor_copy(out=out_ap, in_=in_ap)
                else:
                    nc.scalar.copy(out_ap, in_ap)
                _cp[0] += 1

            # ---- bulk input loads (line-rate, few big DMAs) ----
            # Spread descriptor generation across the three DMA-capable
            # engines (sync/scalar HWDGE + gpsimd SWDGE) so input transfers
            # start within ~1-2us instead of serializing ~12us on one ring.
            MH = M_TILES // 2
            x_stages = []
            for xi, eng in zip(range(2), (nc.sync, nc.scalar)):
                x_st = big_pool.tile([P, MH, L, D], f32, name=f"x_stage{xi}")
                eng.dma_start(
                    x_st,
                    x_dram[xi * MH * P : (xi + 1) * MH * P].rearrange(
                        "(mo bi) p d -> bi mo p d", bi=P
                    ),
                )
                x_stages.append(x_st)
            CO = C_TILES // W_CHUNKS  # 8 c-tiles per chunk
            w_stages = []
            w_engs = (nc.sync, nc.scalar, nc.gpsimd, nc.gpsimd)
            for ci in range(W_CHUNKS):
                w_st = big_pool.tile([P, CO, L, D], f32, name=f"w_stage{ci}")
                w_engs[ci].dma_start(
                    w_st,
                    w_dram[ci * CO * P : (ci + 1) * CO * P].rearrange(
                        "(co bi) p d -> bi co p d", bi=P
                    ),
                )
                w_stages.append(w_st)

            # ---- transpose prologue ----
            # X.T: xt[d, p, b] = X[b, p, d]; 4 transposes per PSUM bank,
            # then one [128, 512] strided copy out.
            xt = big_pool.tile([P, L, B_LOC], f32r, name="xt")
            for mo2 in range(M_TILES // 2):
                x_st = x_stages[mo2 * 2 // MH]
                mo_base = (mo2 * 2) % MH
                ps = psum_t.tile([P, 2, L, P], f32, tag="tps", name="tps_x")
                for m1 in range(2):
                    for p in range(L):
                        nc.tensor.transpose(
                            ps[:, m1, p, :], x_st[:, mo_base + m1, p, :], ident
                        )
                copy(
                    xt[:, :, mo2 * 2 * P : (mo2 * 2 + 2) * P].rearrange(
                        "d p (m b) -> d p m b", m=2
                    ),
                    ps.rearrange("d m p b -> d p m b"),
                )

            # W.T in two halves so early matmuls start before all of W lands:
            # wt_halves[h][d, p, c'] = W[h*2048 + c', p, d]
            wt_halves = [
                big_pool.tile([P, L, C_HALF], f32r, name=f"wt{h}") for h in range(2)
            ]
            for ci in range(W_CHUNKS):
                w_st = w_stages[ci]
                wt = wt_halves[ci // 2]
                base = (ci % 2) * CO * P  # c offset within the half
                for co2 in range(CO // 2):
                    ps = psum_t.tile([P, 2, L, P], f32, tag="tps", name="tps_w")
                    for m1 in range(2):
                        for p in range(L):
                            nc.tensor.transpose(
                                ps[:, m1, p, :], w_st[:, co2 * 2 + m1, p, :], ident
                            )
                    copy(
                        wt[
                            :, :, base + co2 * 2 * P : base + (co2 * 2 + 2) * P
                        ].rearrange("d p (m b) -> d p m b", m=2),
                        ps.rearrange("d m p b -> d p m b"),
                    )

            # ---- main matmul stream ----
            for mt in range(M_TILES):
                out_sb = out_pool.tile([P, C], f32, tag="osb", name="out_sb")
                for ng in range(N_TILES // N_GROUP):
                    pms = [
                        psum_mm.tile([P, N_TILE], f32, tag="pmm", name="pmm")
                        for _ in range(N_GROUP)
                    ]
                    # p outer: stationary operand xt slice reused across the
                    # N_GROUP matmuls (amortizes weight load).
                    for p in range(L):
                        for j in range(N_GROUP):
                            nt = ng * N_GROUP + j
                            wt = wt_halves[nt * N_TILE // C_HALF]
                            noff = nt * N_TILE % C_HALF
                            nc.tensor.matmul(
                                pms[j],
                                xt[:, p, mt * P : (mt + 1) * P],
                                wt[:, p, noff : noff + N_TILE],
                                start=(p == 0),
                                stop=(p == L - 1),
                            )
                    for j in range(N_GROUP):
                        nt = ng * N_GROUP + j
                        copy(out_sb[:, nt * N_TILE : (nt + 1) * N_TILE], pms[j])
                nc.sync.dma_start(out_dram[mt * P : (mt + 1) * P], out_sb)

    nc.compile()
    return nc


def kernel(batchs, label2embed):
    global LAST_RESULT
    from concourse.bass_utils import run_bass_kernel_spmd

    if "nc" not in _CACHE:
        _CACHE["nc"] = _build()
    nc = _CACHE["nc"]

    batchs = np.ascontiguousarray(batchs, dtype=np.float32)
    label2embed = np.ascontiguousarray(label2embed, dtype=np.float32)
    assert batchs.shape == (B, L, D) and label2embed.shape == (C, L, D)

    in_maps = [
        {
            "batchs": batchs[c * B_LOC : (c + 1) * B_LOC],
            "label2embed": label2embed,
        }
        for c in range(N_CORES)
    ]
    res = run_bass_kernel_spmd(
        nc,
        in_maps,
        core_ids=list(range(N_CORES)),
        trace=PROFILE,
        trace_cores=list(range(N_CORES)) if (PROFILE and TRACE_ALL_CORES) else None,
    )
    LAST_RESULT = res
    return np.concatenate([r["out"] for r in res.results], axis=0)


# revision 7
# speedup vs baseline: 1.1348x; 1.1348x over previous
"""Trainium2 Bass kernel for nn_ETypePromptModel: logits = einsum('bpd,cpd->bc').

Equivalent to X @ W.T with X=[B, L*D]=[16384, 256], W=[C, L*D]=[4096, 256].
Data-parallel over B across 8 NeuronCores; label2embed replicated.

Per-core plan (B_LOC=2048):
  - Load X (1 DMA, 2 MB) and W (4 DMAs, 4 MB) up front at line rate.
  - PE-transpose X (32 tiles) and W (64 tiles) into K-major float32r SBUF
    layout; 4 transposes batched per PSUM bank -> one [128,512] copy each.
  - 256 float32r matmuls ([128k x 128b] stationary, [128k x 512c] moving),
    K=256 accumulated over 2 PSUM passes, 6 PSUM banks in rotation.
  - PSUM -> SBUF copies alternate Vector/Scalar engines; 16 x 2MB HWDGE
    DMA writes of the [2048, 4096] fp32 output slice.
"""

import sys

import numpy as np

sys.path.insert(0, "/opt/trn_rl_repo")

B, C, L, D = 16384, 4096, 2, 128
N_CORES = 8
B_LOC = B // N_CORES  # 2048
P = 128
N_TILE = 512  # moving free dim per matmul
M_TILES = B_LOC // P  # 16
N_TILES = C // N_TILE  # 8
C_TILES = C // P  # 32
N_GROUP = 4  # matmul accumulation group width (PSUM banks)
W_CHUNKS = 4  # input-load chunks for W
C_HALF = C // 2

_CACHE = {}
PROFILE = False
TRACE_ALL_CORES = False
LAST_RESULT = None


def _build():
    import concourse.mybir as mybir
    import concourse.tile as tile
    from concourse import bacc
    from concourse.masks import make_identity

    f32 = mybir.dt.float32
    f32r = mybir.dt.float32r

    nc = bacc.Bacc(
        "TRN2",
        target_bir_lowering=False,
        debug=False,
        enable_asserts=False,
        num_devices=N_CORES,
    )

    x_dram = nc.dram_tensor("batchs", [B_LOC, L, D], f32, kind="ExternalInput").ap()
    w_dram = nc.dram_tensor("label2embed", [C, L, D], f32, kind="ExternalInput").ap()
    out_dram = nc.dram_tensor("out", [B_LOC, C], f32, kind="ExternalOutput").ap()

    with tile.TileContext(nc) as tc:
        with (
            tc.tile_pool(name="const", bufs=1) as const_pool,
            tc.tile_pool(name="big", bufs=1) as big_pool,
            tc.tile_pool(name="osb", bufs=4) as out_pool,
            tc.tile_pool(name="pst", bufs=2, space="PSUM") as psum_t,
            tc.tile_pool(name="psm", bufs=6, space="PSUM") as psum_mm,
        ):
            ident = const_pool.tile([P, P], f32, name="ident")
            make_identity(nc, ident)

            _cp = [0]

            def copy(out_ap, in_ap):
                if _cp[0] % 2 == 0:
                    nc.vector.tensor_copy(out=out_ap, in_=in_ap)
                else:
                    nc.scalar.copy(out_ap, in_ap)
                _cp[0] += 1

            # ---- bulk input loads (line-rate, few big DMAs) ----
            # Spread descriptor generation across the three DMA-capable
            # engines (sync/scalar HWDGE + gpsimd SWDGE) so input transfers
            # start within ~1-2us instead of serializing ~12us on one ring.
            MH = M_TILES // 2
            x_stages = []
            for xi, eng in zip(range(2), (nc.sync, nc.scalar)):
                x_st = big_pool.tile([P, MH, L, D], f32, name=f"x_stage{xi}")
                eng.dma_start(
                    x_st,
                    x_dram[xi * MH * P : (xi + 1) * MH * P].rearrange(
                        "(mo bi) p d -> bi mo p d", bi=P
                    ),
                )
                x_stages.append(x_st)
            CO = C_TILES // W_CHUNKS  # 8 c-tiles per chunk
            w_stages = []
            w_engs = (nc.sync, nc.scalar, nc.sync, nc.scalar)
            for ci in range(W_CHUNKS):
                w_st = big_pool.tile([P, CO, L, D], f32, name=f"w_stage{ci}")
                w_engs[ci].dma_start(
                    w_st,
                    w_dram[ci * CO * P : (ci + 1) * CO * P].rearrange(
                        "(co bi) p d -> bi co p d", bi=P
                    ),
                )
                w_stages.append(w_st)

            # ---- transpose prologue ----
            # X.T: xt[d, p, b] = X[b, p, d]; 4 transposes per PSUM bank,
            # then one [128, 512] strided copy out.
            xt = big_pool.tile([P, L, B_LOC], f32r, name="xt")
            for mo2 in range(M_TILES // 2):
                x_st = x_stages[mo2 * 2 // MH]
                mo_base = (mo2 * 2) % MH
                ps = psum_t.tile([P, 2, L, P], f32, tag="tps", name="tps_x")
                for m1 in range(2):
                    for p in range(L):
                        nc.tensor.transpose(
                            ps[:, m1, p, :], x_st[:, mo_base + m1, p, :], ident
                        )
                copy(
                    xt[:, :, mo2 * 2 * P : (mo2 * 2 + 2) * P].rearrange(
                        "d p (m b) -> d p m b", m=2
                    ),
                    ps.rearrange("d m p b -> d p m b"),
                )

            # W.T in two halves so early matmuls start before all of W lands:
            # wt_halves[h][d, p, c'] = W[h*2048 + c', p, d]
            wt_halves = [
                big_pool.tile([P, L, C_HALF], f32r, name=f"wt{h}") for h in range(2)
            ]
            for ci in range(W_CHUNKS):
                w_st = w_stages[ci]
                wt = wt_halves[ci // 2]
                base = (ci % 2) * CO * P  # c offset within the half
                for co2 in range(CO // 2):
                    ps = psum_t.tile([P, 2, L, P], f32, tag="tps", name="tps_w")
                    for m1 in range(2):
                        for p in range(L):
                            nc.tensor.transpose(
                                ps[:, m1, p, :], w_st[:, co2 * 2 + m1, p, :], ident
                            )
                    copy(
                        wt[
                            :, :, base + co2 * 2 * P : base + (co2 * 2 + 2) * P
                        ].rearrange("d p (m b) -> d p m b", m=2),
                        ps.rearrange("d m p b -> d p m b"),
                    )

            # ---- main matmul stream ----
            for mt in range(M_TILES):
                out_sb = out_pool.tile([P, C], f32, tag="osb", name="out_sb")
                for ng in range(N_TILES // N_GROUP):
                    pms = [
                        psum_mm.tile([P, N_TILE], f32, tag="pmm", name="pmm")
                        for _ in range(N_GROUP)
                    ]
                    # p outer: stationary operand xt slice reused across the
                    # N_GROUP matmuls (amortizes weight load).
                    for p in range(L):
                        for j in range(N_GROUP):
                            nt = ng * N_GROUP + j
                            wt = wt_halves[nt * N_TILE // C_HALF]
                            noff = nt * N_TILE % C_HALF
                            nc.tensor.matmul(
                                pms[j],
                                xt[:, p, mt * P : (mt + 1) * P],
                                wt[:, p, noff : noff + N_TILE],
                                start=(p == 0),
                                stop=(p == L - 1),
                            )
                    for j in range(N_GROUP):
                        nt = ng * N_GROUP + j
                        copy(out_sb[:, nt * N_TILE : (nt + 1) * N_TILE], pms[j])
                nc.sync.dma_start(out_dram[mt * P : (mt + 1) * P], out_sb)

    nc.compile()
    return nc


def kernel(batchs, label2embed):
    global LAST_RESULT
    from concourse.bass_utils import run_bass_kernel_spmd

    if "nc" not in _CACHE:
        _CACHE["nc"] = _build()
    nc = _CACHE["nc"]

    batchs = np.ascontiguousarray(batchs, dtype=np.float32)
    label2embed = np.ascontiguousarray(label2embed, dtype=np.float32)
    assert batchs.shape == (B, L, D) and label2embed.shape == (C, L, D)

    in_maps = [
        {
            "batchs": batchs[c * B_LOC : (c + 1) * B_LOC],
            "label2embed": label2embed,
        }
        for c in range(N_CORES)
    ]
    res = run_bass_kernel_spmd(
        nc,
        in_maps,
        core_ids=list(range(N_CORES)),
        trace=PROFILE,
        trace_cores=list(range(N_CORES)) if (PROFILE and TRACE_ALL_CORES) else None,
    )
    LAST_RESULT = res
    return np.concatenate([r["out"] for r in res.results], axis=0)


# revision 8
# speedup vs baseline: 1.1937x; 1.0518x over previous
"""Trainium2 Bass kernel for nn_ETypePromptModel: logits = einsum('bpd,cpd->bc').

Equivalent to X @ W.T with X=[B, L*D]=[16384, 256], W=[C, L*D]=[4096, 256].
Data-parallel over B across 8 NeuronCores; label2embed replicated.

Per-core plan (B_LOC=2048):
  - Load X (2 DMAs) and W (4 DMAs) up front, descriptor-gen split across
    both HWDGE rings (sync + scalar) so transfers start early.
  - PE-transpose X and W into K-major float32r SBUF layout (fp32 has no
    DMA-transpose path); 4 transposes batched per PSUM bank -> one
    [128,512] strided copy each.
  - W handled in 4 chunks of 1024 classes; the matmul stream is
    chunk-outer so the first output DMA fires as soon as chunk 0 is
    transposed (~15us), and later chunks' transposes interleave into the
    matmul stream.
  - 256 float32r matmuls ([128k x 128b] stationary, [128k x 512c] moving),
    K=256 accumulated over 2 PSUM passes; groups of 2 PSUM banks, 3 groups
    in flight.
  - PSUM -> SBUF copies alternate Vector/Scalar engines; 64 x 512KB HWDGE
    DMA writes of the [2048, 4096] fp32 output slice.
"""

import sys

import numpy as np

sys.path.insert(0, "/opt/trn_rl_repo")

B, C, L, D = 16384, 4096, 2, 128
N_CORES = 8
B_LOC = B // N_CORES  # 2048
P = 128
N_TILE = 512  # moving free dim per matmul
M_TILES = B_LOC // P  # 16
C_TILES = C // P  # 32
W_CHUNKS = 4
C_CHUNK = C // W_CHUNKS  # 1024 classes per chunk
N_GROUP = 2  # PSUM banks per matmul accumulation group

_CACHE = {}
PROFILE = False
TRACE_ALL_CORES = False
LAST_RESULT = None


def _build():
    import concourse.mybir as mybir
    import concourse.tile as tile
    from concourse import bacc
    from concourse.masks import make_identity

    f32 = mybir.dt.float32
    f32r = mybir.dt.float32r

    nc = bacc.Bacc(
        "TRN2",
        target_bir_lowering=False,
        debug=False,
        enable_asserts=False,
        num_devices=N_CORES,
    )

    x_dram = nc.dram_tensor("batchs", [B_LOC, L, D], f32, kind="ExternalInput").ap()
    w_dram = nc.dram_tensor("label2embed", [C, L, D], f32, kind="ExternalInput").ap()
    out_dram = nc.dram_tensor("out", [B_LOC, C], f32, kind="ExternalOutput").ap()

    with tile.TileContext(nc) as tc:
        with (
            tc.tile_pool(name="const", bufs=1) as const_pool,
            tc.tile_pool(name="big", bufs=1) as big_pool,
            tc.tile_pool(name="osb", bufs=8) as out_pool,
            tc.tile_pool(name="pst", bufs=2, space="PSUM") as psum_t,
            tc.tile_pool(name="psm", bufs=6, space="PSUM") as psum_mm,
        ):
            ident = const_pool.tile([P, P], f32, name="ident")
            make_identity(nc, ident)

            _cp = [0]

            def copy(out_ap, in_ap):
                if _cp[0] % 2 == 0:
                    nc.vector.tensor_copy(out=out_ap, in_=in_ap)
                else:
                    nc.scalar.copy(out_ap, in_ap)
                _cp[0] += 1

            # ---- bulk input loads ----
            MH = M_TILES // 2
            x_stages = []
            for xi, eng in zip(range(2), (nc.sync, nc.scalar)):
                x_st = big_pool.tile([P, MH, L, D], f32, name=f"x_stage{xi}")
                eng.dma_start(
                    x_st,
                    x_dram[xi * MH * P : (xi + 1) * MH * P].rearrange(
                        "(mo bi) p d -> bi mo p d", bi=P
                    ),
                )
                x_stages.append(x_st)
            CO = C_TILES // W_CHUNKS  # 8 c-tiles per chunk
            w_stages = []
            w_engs = (nc.sync, nc.scalar, nc.sync, nc.scalar)
            for ci in range(W_CHUNKS):
                w_st = big_pool.tile([P, CO, L, D], f32, name=f"w_stage{ci}")
                w_engs[ci].dma_start(
                    w_st,
                    w_dram[ci * CO * P : (ci + 1) * CO * P].rearrange(
                        "(co bi) p d -> bi co p d", bi=P
                    ),
                )
                w_stages.append(w_st)

            # ---- transposes ----
            # 4 [128,128] PE transposes batched into one PSUM bank, then one
            # [128, 2, 2, 128] strided copy out (cast to f32r).
            def transpose_batch(dst, dst_off, src, src_off, tag):
                ps = psum_t.tile([P, 2, L, P], f32, tag="tps", name=tag)
                for m1 in range(2):
                    for p in range(L):
                        nc.tensor.transpose(
                            ps[:, m1, p, :], src[:, src_off + m1, p, :], ident
                        )
                copy(
                    dst[:, :, dst_off : dst_off + 2 * P].rearrange(
                        "d p (m b) -> d p m b", m=2
                    ),
                    ps.rearrange("d m p b -> d p m b"),
                )

            # X.T: xt[d, p, b] = X[b, p, d]
            xt = big_pool.tile([P, L, B_LOC], f32r, name="xt")
            for mo2 in range(M_TILES // 2):
                transpose_batch(
                    xt,
                    mo2 * 2 * P,
                    x_stages[mo2 * 2 // MH],
                    (mo2 * 2) % MH,
                    "tps_x",
                )

            # W.T per chunk: wt_chunks[ci][d, p, c'] = W[ci*1024 + c', p, d]
            wt_chunks = [
                big_pool.tile([P, L, C_CHUNK], f32r, name=f"wt{ci}")
                for ci in range(W_CHUNKS)
            ]

            def w_transpose_batch(ci, co2):
                transpose_batch(
                    wt_chunks[ci], co2 * 2 * P, w_stages[ci], co2 * 2, "tps_w"
                )

            # chunk 0 fully transposed up front; chunks 1..3 interleave below
            for co2 in range(CO // 2):
                w_transpose_batch(0, co2)

            # ---- main matmul stream: chunk-outer ----
            for ci in range(W_CHUNKS):
                wt = wt_chunks[ci]
                for mt in range(M_TILES):
                    # spread next chunk's transposes through this phase
                    if ci < W_CHUNKS - 1 and mt % 4 == 0:
                        w_transpose_batch(ci + 1, mt // 4)

                    out_sb = out_pool.tile([P, C_CHUNK], f32, tag="osb", name="out_sb")
                    pms = [
                        psum_mm.tile([P, N_TILE], f32, tag="pmm", name="pmm")
                        for _ in range(N_GROUP)
                    ]
                    for p in range(L):
                        for j in range(N_GROUP):
                            nc.tensor.matmul(
                                pms[j],
                                xt[:, p, mt * P : (mt + 1) * P],
                                wt[:, p, j * N_TILE : (j + 1) * N_TILE],
                                start=(p == 0),
                                stop=(p == L - 1),
                            )
                    for j in range(N_GROUP):
                        copy(out_sb[:, j * N_TILE : (j + 1) * N_TILE], pms[j])
                    nc.sync.dma_start(
                        out_dram[
                            mt * P : (mt + 1) * P,
                            ci * C_CHUNK : (ci + 1) * C_CHUNK,
                        ],
                        out_sb,
                    )

    nc.compile()
    return nc


def kernel(batchs, label2embed):
    global LAST_RESULT
    from concourse.bass_utils import run_bass_kernel_spmd

    if "nc" not in _CACHE:
        _CACHE["nc"] = _build()
    nc = _CACHE["nc"]

    batchs = np.ascontiguousarray(batchs, dtype=np.float32)
    label2embed = np.ascontiguousarray(label2embed, dtype=np.float32)
    assert batchs.shape == (B, L, D) and label2embed.shape == (C, L, D)

    in_maps = [
        {
            "batchs": batchs[c * B_LOC : (c + 1) * B_LOC],
            "label2embed": label2embed,
        }
        for c in range(N_CORES)
    ]
    res = run_bass_kernel_spmd(
        nc,
        in_maps,
        core_ids=list(range(N_CORES)),
        trace=PROFILE,
        trace_cores=list(range(N_CORES)) if (PROFILE and TRACE_ALL_CORES) else None,
    )
    LAST_RESULT = res
    return np.concatenate([r["out"] for r in res.results], axis=0)


# revision 9
# speedup vs baseline: 1.2091x; 1.0129x over previous
"""Trainium2 Bass kernel for nn_ETypePromptModel: logits = einsum('bpd,cpd->bc').

Equivalent to X @ W.T with X=[B, L*D]=[16384, 256], W=[C, L*D]=[4096, 256].
Data-parallel over B across 8 NeuronCores; label2embed replicated.

Per-core plan (B_LOC=2048):
  - Load X (2 DMAs) and W (4 DMAs) up front, descriptor-gen split across
    both HWDGE rings (sync + scalar) so transfers start early.
  - PE-transpose X and W into K-major float32r SBUF layout (fp32 has no
    DMA-transpose path); 4 transposes batched per PSUM bank -> one
    [128,512] strided copy each.
  - W handled in 4 chunks of 1024 classes; the matmul stream is
    chunk-outer so the first output DMA fires as soon as chunk 0 is
    transposed (~15us), and later chunks' transposes interleave into the
    matmul stream.
  - 256 float32r matmuls ([128k x 128b] stationary, [128k x 512c] moving),
    K=256 accumulated over 2 PSUM passes; groups of 2 PSUM banks, 3 groups
    in flight.
  - PSUM -> SBUF copies alternate Vector/Scalar engines; 64 x 512KB HWDGE
    DMA writes of the [2048, 4096] fp32 output slice.
"""

import sys

import numpy as np

sys.path.insert(0, "/opt/trn_rl_repo")

B, C, L, D = 16384, 4096, 2, 128
N_CORES = 8
B_LOC = B // N_CORES  # 2048
P = 128
N_TILE = 512  # moving free dim per matmul
M_TILES = B_LOC // P  # 16
C_TILES = C // P  # 32
W_CHUNKS = 4
C_CHUNK = C // W_CHUNKS  # 1024 classes per chunk
N_GROUP = 2  # PSUM banks per matmul accumulation group

_CACHE = {}
PROFILE = False
TRACE_ALL_CORES = False
LAST_RESULT = None


def _build():
    import concourse.mybir as mybir
    import concourse.tile as tile
    from concourse import bacc
    from concourse.masks import make_identity

    f32 = mybir.dt.float32
    f32r = mybir.dt.float32r

    nc = bacc.Bacc(
        "TRN2",
        target_bir_lowering=False,
        debug=False,
        enable_asserts=False,
        num_devices=N_CORES,
    )

    x_dram = nc.dram_tensor("batchs", [B_LOC, L, D], f32, kind="ExternalInput").ap()
    w_dram = nc.dram_tensor("label2embed", [C, L, D], f32, kind="ExternalInput").ap()
    out_dram = nc.dram_tensor("out", [B_LOC, C], f32, kind="ExternalOutput").ap()

    with tile.TileContext(nc) as tc:
        with (
            tc.tile_pool(name="const", bufs=1) as const_pool,
            tc.tile_pool(name="big", bufs=1) as big_pool,
            tc.tile_pool(name="osb", bufs=8) as out_pool,
            tc.tile_pool(name="pst", bufs=2, space="PSUM") as psum_t,
            tc.tile_pool(name="psm", bufs=6, space="PSUM") as psum_mm,
        ):
            ident = const_pool.tile([P, P], f32, name="ident")
            make_identity(nc, ident)

            _cp = [0]

            def copy(out_ap, in_ap):
                if _cp[0] % 2 == 0:
                    nc.vector.tensor_copy(out=out_ap, in_=in_ap)
                else:
                    nc.scalar.copy(out_ap, in_ap)
                _cp[0] += 1

            # ---- bulk input loads ----
            # X first on both HWDGE rings (4 chunks of 4 m-tiles), then W
            # chunks 0/1; W chunks 2/3 are triggered mid-stream so early DMA
            # bandwidth goes to the data the pipeline start needs.
            XQ = 4  # m-tiles per X chunk
            x_stages = []
            x_engs = (nc.sync, nc.scalar, nc.sync, nc.scalar)
            for xi in range(M_TILES // XQ):
                x_st = big_pool.tile([P, XQ, L, D], f32, name=f"x_stage{xi}")
                x_engs[xi].dma_start(
                    x_st,
                    x_dram[xi * XQ * P : (xi + 1) * XQ * P].rearrange(
                        "(mo bi) p d -> bi mo p d", bi=P
                    ),
                )
                x_stages.append(x_st)
            CO = C_TILES // W_CHUNKS  # 8 c-tiles per chunk
            w_engs = (nc.sync, nc.scalar, nc.sync, nc.scalar)
            w_stages = [
                big_pool.tile([P, CO, L, D], f32, name=f"w_stage{ci}")
                for ci in range(W_CHUNKS)
            ]

            def load_w_chunk(ci):
                w_engs[ci].dma_start(
                    w_stages[ci],
                    w_dram[ci * CO * P : (ci + 1) * CO * P].rearrange(
                        "(co bi) p d -> bi co p d", bi=P
                    ),
                )

            load_w_chunk(0)
            load_w_chunk(1)

            # ---- transposes ----
            # 4 [128,128] PE transposes batched into one PSUM bank, then one
            # [128, 2, 2, 128] strided copy out (cast to f32r).
            def transpose_batch(dst, dst_off, src, src_off, tag):
                ps = psum_t.tile([P, 2, L, P], f32, tag="tps", name=tag)
                for m1 in range(2):
                    for p in range(L):
                        nc.tensor.transpose(
                            ps[:, m1, p, :], src[:, src_off + m1, p, :], ident
                        )
                copy(
                    dst[:, :, dst_off : dst_off + 2 * P].rearrange(
                        "d p (m b) -> d p m b", m=2
                    ),
                    ps.rearrange("d m p b -> d p m b"),
                )

            # X.T per chunk: xt_chunks[q][d, p, b'] = X[q*512 + b', p, d]
            xt_chunks = [
                big_pool.tile([P, L, XQ * P], f32r, name=f"xt{xi}")
                for xi in range(M_TILES // XQ)
            ]
            for mo2 in range(M_TILES // 2):
                xi = mo2 * 2 // XQ
                transpose_batch(
                    xt_chunks[xi],
                    ((mo2 * 2) % XQ) * P,
                    x_stages[xi],
                    (mo2 * 2) % XQ,
                    "tps_x",
                )

            # W.T per chunk: wt_chunks[ci][d, p, c'] = W[ci*1024 + c', p, d]
            wt_chunks = [
                big_pool.tile([P, L, C_CHUNK], f32r, name=f"wt{ci}")
                for ci in range(W_CHUNKS)
            ]

            def w_transpose_batch(ci, co2):
                transpose_batch(
                    wt_chunks[ci], co2 * 2 * P, w_stages[ci], co2 * 2, "tps_w"
                )

            # chunk 0 fully transposed up front; chunks 1..3 interleave below
            for co2 in range(CO // 2):
                w_transpose_batch(0, co2)

            # ---- main matmul stream: chunk-outer ----
            for ci in range(W_CHUNKS):
                wt = wt_chunks[ci]
                for mt in range(M_TILES):
                    # trigger deferred W loads once the pipe is rolling
                    if ci == 0 and mt == 2:
                        load_w_chunk(2)
                    if ci == 0 and mt == 6:
                        load_w_chunk(3)
                    # spread next chunk's transposes through this phase
                    if ci < W_CHUNKS - 1 and mt % 4 == 0 and not (ci == 0 and mt < 4):
                        w_transpose_batch(ci + 1, mt // 4)
                    if ci == 0 and mt == 4:
                        w_transpose_batch(1, 0)

                    out_sb = out_pool.tile([P, C_CHUNK], f32, tag="osb", name="out_sb")
                    pms = [
                        psum_mm.tile([P, N_TILE], f32, tag="pmm", name="pmm")
                        for _ in range(N_GROUP)
                    ]
                    for p in range(L):
                        for j in range(N_GROUP):
                            nc.tensor.matmul(
                                pms[j],
                                xt_chunks[mt // XQ][:, p, (mt % XQ) * P : (mt % XQ + 1) * P],
                                wt[:, p, j * N_TILE : (j + 1) * N_TILE],
                                start=(p == 0),
                                stop=(p == L - 1),
                            )
                    for j in range(N_GROUP):
                        copy(out_sb[:, j * N_TILE : (j + 1) * N_TILE], pms[j])
                    nc.sync.dma_start(
                        out_dram[
                            mt * P : (mt + 1) * P,
                            ci * C_CHUNK : (ci + 1) * C_CHUNK,
                        ],
                        out_sb,
                    )

    nc.compile()
    return nc


def kernel(batchs, label2embed):
    global LAST_RESULT
    from concourse.bass_utils import run_bass_kernel_spmd

    if "nc" not in _CACHE:
        _CACHE["nc"] = _build()
    nc = _CACHE["nc"]

    batchs = np.ascontiguousarray(batchs, dtype=np.float32)
    label2embed = np.ascontiguousarray(label2embed, dtype=np.float32)
    assert batchs.shape == (B, L, D) and label2embed.shape == (C, L, D)

    in_maps = [
        {
            "batchs": batchs[c * B_LOC : (c + 1) * B_LOC],
            "label2embed": label2embed,
        }
        for c in range(N_CORES)
    ]
    res = run_bass_kernel_spmd(
        nc,
        in_maps,
        core_ids=list(range(N_CORES)),
        trace=PROFILE,
        trace_cores=list(range(N_CORES)) if (PROFILE and TRACE_ALL_CORES) else None,
    )
    LAST_RESULT = res
    return np.concatenate([r["out"] for r in res.results], axis=0)
